# revision 11
# baseline (speedup 1.0000x reference)
"""BitLinear (1-bit packed weights) on 8 TRN2 NeuronCores.

out = x @ W.T, x [64, 4096] f32, W [11008, 4096] in {-1,+1} unpacked from
bp (one byte per int32, MSB-first bits).

Strategy (tensor-parallel, no collectives):
 - shard out_features 11008 -> 8 x 1376 rows of W; x replicated.
 - host: repack bp bytes into dense 16-bit words (pure bit layout change),
   transposed to [word-idx, n] and laid out as one [128, 2752] tile per
   core (both 128-word chunks side by side); permute x to match.
 - device per core (raw Block, manual semaphores):
     DVE: w1 = words & (1<<s)            (one op extracts BOTH chunks' plane)
     DVE/ACT: u = w1 * 2^(1-s) - 1       (arith + cast -> exact {-1,+1} bf16)
     PE: column-tiled pairs - chunk (c=0,o) on array cols 0-63 -> psum
         partitions 0-63, chunk (c=1,o) on cols 64-127 -> partitions 64-127,
         running concurrently; accumulate over o=0..15.
     DVE: merge psum[0:64] + psum[64:128] -> out tile; DMA out.
 - PE warmup: dummy matmuls during the input-DMA wait so HAM un-throttles
   before the real accumulation starts.
"""

import sys

sys.path.insert(0, "/opt/trn_rl_repo")

import ml_dtypes
import numpy as np

import concourse.bass as bass
import concourse.mybir as mybir
from concourse.bass_utils import run_bass_kernel_spmd

OUT_F = 11008
IN_F = 4096
M = 64
NCORES = 8
NSH = OUT_F // NCORES  # 1376 rows of W per core
NSH2 = 2 * NSH  # pair-tile width (both chunks)

PACK = 16  # bits per packed word on device
NW = IN_F // PACK  # packed words along k per W row (256)
NCH = NW // 128  # 128-partition word chunks (2)
NPAIR = PACK  # 16 plane-pairs (o = bit offset in word)
NSPLITS = (512, 512, 352)  # psum n-chunks (bank = 512 f32)

_dt_word = {16: mybir.dt.uint16, 32: mybir.dt.uint32}[PACK]
_np_word = {16: "<u2", 32: "<u4"}[PACK]

B1 = 2  # w1 pair buffer depth
B2 = 4  # u pair buffer depth
N_WARMUP = 26  # dummy PE matmuls (N=512) to trip the HAM un-throttle
ACT_CAST = frozenset({1, 3, 5, 7, 9, 11, 13, 15})  # pairs whose cast runs on ACT


def _shift(o):
    # word bit position holding k-offset o (little-endian byte packing,
    # MSB-first bit order inside each byte)
    return 8 * (o // 8) + 7 - (o % 8)


def _build():
    nc = bass.Bass()
    bpt = nc.declare_dram_parameter("bpt", [128, NSH2], _dt_word, isOutput=False)
    xr = nc.declare_dram_parameter(
        "xr", [128, (IN_F // 128) * M], mybir.dt.bfloat16, isOutput=False
    )
    out = nc.declare_dram_parameter("out", [M, NSH], mybir.dt.float32, isOutput=True)

    A = mybir.AluOpType

    # engine program-order bookkeeping
    dve_idx = {}  # ('and'|'cast', o) -> 1-based completion count on DVE
    act_idx = {}  # o -> 1-based completion count on ACT
    di = 0
    ai = 0
    for o in range(NPAIR):
        di += 1
        dve_idx[("and", o)] = di
        if o in ACT_CAST:
            ai += 1
            act_idx[o] = ai
        else:
            di += 1
            dve_idx[("cast", o)] = di

    with (
        nc.sbuf_tensor("xb", [128, (IN_F // 128) * M], mybir.dt.bfloat16) as xb,
        nc.sbuf_tensor("btw", [128, NSH2], _dt_word) as btw,
        nc.sbuf_tensor("w1", [128, B1, NSH2], _dt_word) as w1,
        nc.sbuf_tensor("u", [128, B2, NSH2], mybir.dt.bfloat16) as u,
        nc.sbuf_tensor("ot", [M, NSH], mybir.dt.float32) as ot,
        nc.sbuf_tensor("otB", [M, NSH], mybir.dt.float32) as otB,
        nc.sbuf_tensor("junk", [128, 512], mybir.dt.bfloat16) as junk,
        nc.sbuf_tensor("scr", [1, 1], mybir.dt.float32) as scr,
        nc.psum_tensor("ps0", [128, NSPLITS[0]], mybir.dt.float32) as ps0,
        nc.psum_tensor("ps1", [128, NSPLITS[1]], mybir.dt.float32) as ps1,
        nc.psum_tensor("ps2", [128, NSPLITS[2]], mybir.dt.float32) as ps2,
        nc.psum_tensor("psw", [M, 512], mybir.dt.float32) as psw,
        nc.semaphore("sq") as sq,
        nc.semaphore("sb") as sb,
        nc.semaphore("sv") as sv,
        nc.semaphore("sa") as sa,
        nc.semaphore("sp") as sp,
        nc.semaphore("scp") as scp,
        nc.semaphore("so") as so,
        nc.semaphore("sdone") as sdone,
        nc.Block() as block,
    ):
        pss = [ps0, ps1, ps2]

        @block.sync
        def _(sync: bass.BassEngine):
            sync.dma_start(out=btw[:, :], in_=bpt[:, :]).then_inc(sb, 16)
            sync.dma_start(out=xb[:, :], in_=xr[:, :]).then_inc(sq, 16)
            # output: wait for the 3 psum merges, DMA out, then cleanup
            sync.wait_ge(scp, 3)
            sync.dma_start(out=out[:, :], in_=ot[:, :]).then_inc(so, 16)
            sync.wait_ge(so, 16)
            sync.wait_ge(sdone, 3)
            for s in (sq, sb, sv, sa, sp, scp, so, sdone):
                sync.sem_clear(s)

        @block.vector
        def _(vector: bass.BassEngine):
            vector.wait_ge(sb, 16)
            for o in range(NPAIR):
                s = _shift(o)
                # w1 slot free? its reader is cast(o-B1)
                if o >= B1 and (o - B1) in ACT_CAST:
                    vector.wait_ge(sa, act_idx[o - B1])
                vector.tensor_scalar(
                    w1[:, o % B1, :], btw[:, :], 1 << s, None, op0=A.bitwise_and
                ).then_inc(sv)
                if o not in ACT_CAST:
                    if o >= B2:
                        vector.wait_ge(sp, o - B2 + 1)
                    vector.tensor_scalar(
                        u[:, o % B2, :],
                        w1[:, o % B1, :],
                        float(2.0 ** (1 - s)),
                        -1.0,
                        op0=A.mult,
                        op1=A.add,
                    ).then_inc(sv)
            # merge the two column-tile halves of each psum bank
            # (ACT first copies the high half to SBUF; TT can read only
            # one PSUM operand)
            off = 0
            for j, w in enumerate(NSPLITS):
                vector.wait_ge(sa, len(ACT_CAST) + j + 1)
                vector.tensor_tensor(
                    ot[:, off : off + w],
                    pss[j][0:M, :],
                    otB[:, off : off + w],
                    op=A.add,
                ).then_inc(scp)
                off += w
            vector.nop().then_inc(sdone)

        @block.scalar
        def _(scalar: bass.BassEngine):
            # touch the ACT path early so the activation table loads during
            # the DMA wait instead of on the first real cast
            scalar.activation(
                scr[:, :], scr[:, :], mybir.ActivationFunctionType.Copy, 0.0, 0.0
            )
            for o in sorted(ACT_CAST):
                s = _shift(o)
                scalar.wait_ge(sv, dve_idx[("and", o)])
                if o >= B2:
                    scalar.wait_ge(sp, o - B2 + 1)
                scalar.activation(
                    u[:, o % B2, :],
                    w1[:, o % B1, :],
                    mybir.ActivationFunctionType.Copy,
                    bias=-1.0,
                    scale=float(2.0 ** (1 - s)),
                ).then_inc(sa)
            # copy the high column-tile halves PSUM -> SBUF for the merge
            scalar.wait_ge(sp, NPAIR)
            off = 0
            for j, w in enumerate(NSPLITS):
                scalar.activation(
                    otB[:, off : off + w],
                    pss[j][M : 2 * M, :],
                    mybir.ActivationFunctionType.Copy,
                    bias=0.0,
                    scale=1.0,
                ).then_inc(sa)
                off += w
            scalar.nop().then_inc(sdone)

        @block.tensor
        def _(tensor: bass.BassEngine):
            # HAM warmup on junk data (no DMA dependency)
            for _i in range(N_WARMUP):
                tensor.matmul(
                    psw[:, :], junk[:, 0:M], junk[:, :], start=True, stop=True
                )
            tensor.wait_ge(sq, 16)
            for o in range(NPAIR):
                if o in ACT_CAST:
                    tensor.wait_ge(sa, act_idx[o])
                else:
                    tensor.wait_ge(sv, dve_idx[("cast", o)])
                st, sp_ = o == 0, o == NPAIR - 1
                lhA = xb[:, o * M : (o + 1) * M]
                lhB = xb[:, (PACK + o) * M : (PACK + o + 1) * M]
                off = 0
                for j, w in enumerate(NSPLITS):
                    tensor.matmul(
                        pss[j][0:M, :],
                        lhA,
                        u[:, o % B2, off : off + w],
                        start=st,
                        stop=sp_,
                        tile_position=(0, 0),
                    )
                    ins = tensor.matmul(
                        pss[j][M : 2 * M, :],
                        lhB,
                        u[:, o % B2, NSH + off : NSH + off + w],
                        start=st,
                        stop=sp_,
                        tile_position=(0, 64),
                    )
                    off += w
                ins.then_inc(sp)
            tensor.nop().then_inc(sdone)

    return nc


def _prep(x, bp):
    x = np.asarray(x, dtype=np.float32)
    bp = np.asarray(bp)
    bytes_ = bp.astype(np.uint8)  # values are 0..255 by construction
    B = bytes_.reshape(OUT_F, IN_F // 8)
    # x[m, k] with k = PACK*(128*c + p) + o  ->  xh[p, (c, o, m)]
    xh = (
        np.ascontiguousarray(x.reshape(M, NCH, 128, PACK).transpose(2, 1, 3, 0))
        .reshape(128, -1)
        .astype(ml_dtypes.bfloat16)
    )
    in_maps = []
    for cid in range(NCORES):
        Bc = np.ascontiguousarray(B[cid * NSH : (cid + 1) * NSH])  # [1376, 512] u8
        Wd = Bc.view(_np_word)  # [1376, NW] little-endian words
        bptT = np.ascontiguousarray(Wd.T)  # [NW=256, 1376]
        # both 128-word chunks side by side: [128, 2752]
        pair = np.concatenate([bptT[0:128, :], bptT[128:256, :]], axis=1)
        in_maps.append({"bpt": np.ascontiguousarray(pair), "xr": xh})
    return in_maps


def _run(x, bp, trace=False):
    in_maps = _prep(x, bp)
    nc = _build()
    res = run_bass_kernel_spmd(nc, in_maps, list(range(NCORES)), trace=trace)
    outs = [np.asarray(res.results[c]["out"]) for c in range(NCORES)]
    full = np.concatenate(outs, axis=1).astype(np.float32)
    return full, res


def kernel(x, bp):
    out, _ = _run(x, bp, trace=False)
    return out


# revision 12
# speedup vs baseline: 1.1799x; 1.1799x over previous
"""BitLinear (1-bit packed weights) on 8 TRN2 NeuronCores.

out = x @ W.T, x [64, 4096] f32, W [11008, 4096] in {-1,+1} unpacked from
bp (one byte per int32, MSB-first bits).

Strategy (tensor-parallel, no collectives):
 - shard out_features 11008 -> 8 x 1376 rows of W; x replicated.
 - host: repack bp bytes into dense 16-bit words (pure bit layout change),
   transposed to [word-idx, n] and laid out as one [128, 2752] tile per
   core (both 128-word chunks side by side); permute x to match.
 - device per core (raw Block, manual semaphores):
     DVE: w1 = words & (1<<s)            (one op extracts BOTH chunks' plane)
     DVE/ACT: u = w1 * 2^(1-s) - 1       (arith + cast -> exact {-1,+1} bf16)
     PE: column-tiled pairs - chunk (c=0,o) on array cols 0-63 -> psum
         partitions 0-63, chunk (c=1,o) on cols 64-127 -> partitions 64-127,
         running concurrently; accumulate over o=0..15.
     DVE: merge psum[0:64] + psum[64:128] -> out tile; DMA out.
 - PE warmup: dummy matmuls during the input-DMA wait so HAM un-throttles
   before the real accumulation starts.
"""

import sys

sys.path.insert(0, "/opt/trn_rl_repo")

import ml_dtypes
import numpy as np

import concourse.bass as bass
import concourse.mybir as mybir
from concourse.bass_utils import run_bass_kernel_spmd

OUT_F = 11008
IN_F = 4096
M = 64
NCORES = 8
NSH = OUT_F // NCORES  # 1376 rows of W per core
NSH2 = 2 * NSH  # pair-tile width (both chunks)

PACK = 16  # bits per packed word on device
NW = IN_F // PACK  # packed words along k per W row (256)
NCH = NW // 128  # 128-partition word chunks (2)
NPAIR = PACK  # 16 plane-pairs (o = bit offset in word)
NSPLITS = (512, 512, 352)  # psum n-chunks (bank = 512 f32)

_dt_word = {16: mybir.dt.uint16, 32: mybir.dt.uint32}[PACK]
_np_word = {16: "<u2", 32: "<u4"}[PACK]

B1 = 3  # w1 pair buffer depth
B2 = 8  # u pair buffer depth
N_WARMUP = 34  # dummy PE matmuls (N=512) to trip the HAM un-throttle
ACT_CAST = frozenset({1, 3, 5, 7, 9, 11, 13, 15})  # pairs whose cast runs on ACT


def _shift(o):
    # word bit position holding k-offset o (little-endian byte packing,
    # MSB-first bit order inside each byte)
    return 8 * (o // 8) + 7 - (o % 8)


def _build():
    nc = bass.Bass()
    bpt = nc.declare_dram_parameter("bpt", [128, NSH2], _dt_word, isOutput=False)
    xr = nc.declare_dram_parameter(
        "xr", [128, (IN_F // 128) * M], mybir.dt.bfloat16, isOutput=False
    )
    out = nc.declare_dram_parameter("out", [M, NSH], mybir.dt.float32, isOutput=True)

    A = mybir.AluOpType

    # engine program-order bookkeeping
    dve_idx = {}  # ('and'|'cast', o) -> 1-based completion count on DVE
    act_idx = {}  # o -> 1-based completion count on ACT
    di = 0
    ai = 0
    for o in range(NPAIR):
        di += 1
        dve_idx[("and", o)] = di
        if o in ACT_CAST:
            ai += 1
            act_idx[o] = ai
        else:
            di += 1
            dve_idx[("cast", o)] = di

    with (
        nc.sbuf_tensor("xb", [128, (IN_F // 128) * M], mybir.dt.bfloat16) as xb,
        nc.sbuf_tensor("btw", [128, NSH2], _dt_word) as btw,
        nc.sbuf_tensor("w1", [128, B1, NSH2], _dt_word) as w1,
        nc.sbuf_tensor("u", [128, B2, NSH2], mybir.dt.bfloat16) as u,
        nc.sbuf_tensor("ot", [M, NSH], mybir.dt.float32) as ot,
        nc.sbuf_tensor("otB", [M, NSH], mybir.dt.float32) as otB,
        nc.sbuf_tensor("junk", [128, 512], mybir.dt.bfloat16) as junk,
        nc.sbuf_tensor("scr", [1, 1], mybir.dt.float32) as scr,
        nc.psum_tensor("ps0", [128, NSPLITS[0]], mybir.dt.float32) as ps0,
        nc.psum_tensor("ps1", [128, NSPLITS[1]], mybir.dt.float32) as ps1,
        nc.psum_tensor("ps2", [128, NSPLITS[2]], mybir.dt.float32) as ps2,
        nc.psum_tensor("psw", [M, 512], mybir.dt.float32) as psw,
        nc.semaphore("sq") as sq,
        nc.semaphore("sb") as sb,
        nc.semaphore("sv") as sv,
        nc.semaphore("sa") as sa,
        nc.semaphore("sp") as sp,
        nc.semaphore("scp") as scp,
        nc.semaphore("so") as so,
        nc.semaphore("sdone") as sdone,
        nc.Block() as block,
    ):
        pss = [ps0, ps1, ps2]

        @block.sync
        def _(sync: bass.BassEngine):
            q = NSH2 // 4
            for i in range(4):
                sync.dma_start(
                    out=btw[:, i * q : (i + 1) * q], in_=bpt[:, i * q : (i + 1) * q]
                ).then_inc(sb, 16)
            xq = (IN_F // 128) * M // 2
            for i in range(2):
                sync.dma_start(
                    out=xb[:, i * xq : (i + 1) * xq], in_=xr[:, i * xq : (i + 1) * xq]
                ).then_inc(sq, 16)
            # output: per-bank pipelined merge -> DMA
            off = 0
            for j, w in enumerate(NSPLITS):
                sync.wait_ge(scp, j + 1)
                sync.dma_start(
                    out=out[:, off : off + w], in_=ot[:, off : off + w]
                ).then_inc(so, 16)
                off += w
            sync.wait_ge(so, 48)
            sync.wait_ge(sdone, 3)
            for s in (sq, sb, sv, sa, sp, scp, so, sdone):
                sync.sem_clear(s)

        @block.vector
        def _(vector: bass.BassEngine):
            vector.wait_ge(sb, 64)
            for o in range(NPAIR):
                s = _shift(o)
                # w1 slot free? its reader is cast(o-B1)
                if o >= B1 and (o - B1) in ACT_CAST:
                    vector.wait_ge(sa, act_idx[o - B1])
                vector.tensor_scalar(
                    w1[:, o % B1, :], btw[:, :], 1 << s, None, op0=A.bitwise_and
                ).then_inc(sv)
                if o not in ACT_CAST:
                    if o >= B2:
                        vector.wait_ge(sp, o - B2 + 1)
                    vector.tensor_scalar(
                        u[:, o % B2, :],
                        w1[:, o % B1, :],
                        float(2.0 ** (1 - s)),
                        -1.0,
                        op0=A.mult,
                        op1=A.add,
                    ).then_inc(sv)
            # merge the two column-tile halves of each psum bank
            # (ACT first copies the high half to SBUF; TT can read only
            # one PSUM operand)
            off = 0
            for j, w in enumerate(NSPLITS):
                vector.wait_ge(sa, len(ACT_CAST) + j + 1)
                vector.tensor_tensor(
                    ot[:, off : off + w],
                    pss[j][0:M, :],
                    otB[:, off : off + w],
                    op=A.add,
                ).then_inc(scp)
                off += w
            vector.nop().then_inc(sdone)

        @block.scalar
        def _(scalar: bass.BassEngine):
            # touch the ACT path early so the activation table loads during
            # the DMA wait instead of on the first real cast
            scalar.activation(
                scr[:, :], scr[:, :], mybir.ActivationFunctionType.Copy, 0.0, 0.0
            )
            for o in sorted(ACT_CAST):
                s = _shift(o)
                scalar.wait_ge(sv, dve_idx[("and", o)])
                if o >= B2:
                    scalar.wait_ge(sp, o - B2 + 1)
                scalar.activation(
                    u[:, o % B2, :],
                    w1[:, o % B1, :],
                    mybir.ActivationFunctionType.Copy,
                    bias=-1.0,
                    scale=float(2.0 ** (1 - s)),
                ).then_inc(sa)
            # copy the high column-tile halves PSUM -> SBUF for the merge
            scalar.wait_ge(sp, NPAIR)
            off = 0
            for j, w in enumerate(NSPLITS):
                scalar.activation(
                    otB[:, off : off + w],
                    pss[j][M : 2 * M, :],
                    mybir.ActivationFunctionType.Copy,
                    bias=0.0,
                    scale=1.0,
                ).then_inc(sa)
                off += w
            scalar.nop().then_inc(sdone)

        @block.tensor
        def _(tensor: bass.BassEngine):
            # HAM warmup on junk data (no DMA dependency)
            for _i in range(N_WARMUP):
                tensor.matmul(
                    psw[:, :], junk[:, 0:M], junk[:, :], start=True, stop=True
                )
            tensor.wait_ge(sq, 32)
            for o in range(NPAIR):
                if o in ACT_CAST:
                    tensor.wait_ge(sa, act_idx[o])
                else:
                    tensor.wait_ge(sv, dve_idx[("cast", o)])
                st, sp_ = o == 0, o == NPAIR - 1
                lhA = xb[:, o * M : (o + 1) * M]
                lhB = xb[:, (PACK + o) * M : (PACK + o + 1) * M]
                off = 0
                for j, w in enumerate(NSPLITS):
                    tensor.matmul(
                        pss[j][0:M, :],
                        lhA,
                        u[:, o % B2, off : off + w],
                        start=st,
                        stop=sp_,
                        tile_position=(0, 0),
                    )
                    ins = tensor.matmul(
                        pss[j][M : 2 * M, :],
                        lhB,
                        u[:, o % B2, NSH + off : NSH + off + w],
                        start=st,
                        stop=sp_,
                        tile_position=(0, 64),
                    )
                    off += w
                ins.then_inc(sp)
            tensor.nop().then_inc(sdone)

    return nc


def _prep(x, bp):
    x = np.asarray(x, dtype=np.float32)
    bp = np.asarray(bp)
    bytes_ = bp.astype(np.uint8)  # values are 0..255 by construction
    B = bytes_.reshape(OUT_F, IN_F // 8)
    # x[m, k] with k = PACK*(128*c + p) + o  ->  xh[p, (c, o, m)]
    xh = (
        np.ascontiguousarray(x.reshape(M, NCH, 128, PACK).transpose(2, 1, 3, 0))
        .reshape(128, -1)
        .astype(ml_dtypes.bfloat16)
    )
    in_maps = []
    for cid in range(NCORES):
        Bc = np.ascontiguousarray(B[cid * NSH : (cid + 1) * NSH])  # [1376, 512] u8
        Wd = Bc.view(_np_word)  # [1376, NW] little-endian words
        bptT = np.ascontiguousarray(Wd.T)  # [NW=256, 1376]
        # both 128-word chunks side by side: [128, 2752]
        pair = np.concatenate([bptT[0:128, :], bptT[128:256, :]], axis=1)
        in_maps.append({"bpt": np.ascontiguousarray(pair), "xr": xh})
    return in_maps


def _run(x, bp, trace=False):
    in_maps = _prep(x, bp)
    nc = _build()
    res = run_bass_kernel_spmd(nc, in_maps, list(range(NCORES)), trace=trace)
    outs = [np.asarray(res.results[c]["out"]) for c in range(NCORES)]
    full = np.concatenate(outs, axis=1).astype(np.float32)
    return full, res


def kernel(x, bp):
    out, _ = _run(x, bp, trace=False)
    return out
